# revision 1
# baseline (speedup 1.0000x reference)
"""CoupledClustersLossV2 Trainium2 kernel.

Full inputs in, full output out. Internally shards the embeddings
[16384, 2048] f32 across 8 NeuronCores along the class axis (each class
owns 64 contiguous rows = 32 pos + 32 neg), computes per-class losses on
each core, and averages on the host.

Per-core algorithm (shard = [2048, 2048] f32 = 32 classes):
  - For each 128-row tile (= 2 classes), one constant 128x128 matrix
    W = I - S (S averages the 32 positive rows of each 64-row class
    block) turns a single TensorEngine matmul into the centered
    residuals  diff = x - anchor  for every row, landing in PSUM.
  - ScalarE squares diff in place and accumulates along the free (D)
    axis -> per-row squared distances d2 [128 rows, 1] per tile.
  - Tail: PE-transpose of the collected d2 [128, 16] -> [16, 128],
    sqrt on ScalarE (+ one Newton step on VectorE), per-class min over
    the negative halves, relu(ap - an + margin), sum of squares ->
    32 per-class losses, DMA'd out as [16, 2].
"""

import sys

import numpy as np

for _p in ("/opt/trn_rl_repo",):
    if _p not in sys.path:
        sys.path.append(_p)

import concourse.bacc as bacc
import concourse.mybir as mybir
from concourse import tile
from concourse.bass_utils import run_bass_kernel_spmd

N_CORES = 8
D = 2048
S = 32                 # samples per class per polarity
ROWS_PER_CLASS = 2 * S # 64: 32 pos then 32 neg
C_PER_CORE = 32        # classes per core (256 / 8)
ROWS_PER_CORE = C_PER_CORE * ROWS_PER_CLASS  # 2048
TILES = ROWS_PER_CORE // 128                 # 16 tiles of 128 rows (2 classes)
SUPER = 8                                    # DMA super-tiles (2 tiles / 2 MiB each)

USE_FP32R = True   # fast fp32 matmul mode (reduced-precision multiply)
MM_DTYPE = "f32r"  # "f32r" | "bf16" (strided truncated-bf16 view of f32 data)
DMA_MODE = "alt"   # "alt" (HWDGE/SWDGE alternating) | "hwdge" | "swdge"
NEWTON = False     # one Newton refinement step after ScalarE sqrt
                   # (HW sqrt measures ~7e-6 rel err; refinement unneeded)
TRACE = False      # set True from test harness for a profiled run

F32 = mybir.dt.float32
F32R = mybir.dt.float32r
AF = mybir.ActivationFunctionType
ALU = mybir.AluOpType

_CACHE = {}
LAST_RESULTS = None


def _w_matrix() -> np.ndarray:
    """W[k, m] so that (W.T @ x)[m] = x[m] - mean(pos rows of m's class)."""
    w = np.zeros((128, 128), dtype=np.float32)
    for b in range(2):  # two 64-row class blocks per tile
        o = 64 * b
        for m in range(64):
            w[o + m, o + m] += 1.0
            w[o : o + S, o + m] -= 1.0 / S
    return w


def _inline_tensor(nc, data: np.ndarray, name: str, dtype):
    """nc.inline_tensor with an explicit BIR dtype (e.g. float32r over
    float32 bytes — same width, so the embedded .npy payload is valid)."""
    import base64
    import io

    import concourse.bass as bass

    data = np.ascontiguousarray(data)
    assert mybir.dt.size(dtype) == data.dtype.itemsize
    mls = nc._tensor(name, list(data.shape), dtype, kind="Const", type="DRAM")
    buf = io.BytesIO()
    np.save(buf, data, allow_pickle=False)
    mls.file = f"{name}.npy"
    mls.ant_data = base64.standard_b64encode(buf.getvalue()).decode()
    return bass.DRamTensorHandle(name, list(data.shape), dtype)


def _build(margin: float, loop_n: int | None = None, stage: str = "full"):
    import ml_dtypes

    nc = bacc.Bacc("TRN2", target_bir_lowering=False, debug=False)
    bf16_mm = MM_DTYPE == "bf16"
    in_dt = F32 if bf16_mm else (F32R if USE_FP32R else F32)
    emb = nc.dram_tensor("emb", [ROWS_PER_CORE, D], in_dt, kind="ExternalInput")
    out = nc.dram_tensor("losses", [TILES, 2], F32, kind="ExternalOutput")

    if bf16_mm:
        w_const = nc.inline_tensor(
            _w_matrix().astype(ml_dtypes.bfloat16), name="wmat"
        )
        w_dt = mybir.dt.bfloat16
    else:
        w_const = _inline_tensor(nc, _w_matrix(), "wmat", in_dt)
        w_dt = in_dt
    id_const = nc.inline_tensor(np.eye(128, dtype=np.float32), name="ident")

    with tile.TileContext(nc) as tc:
        with (
            tc.tile_pool(name="consts", bufs=1) as cpool,
            tc.tile_pool(name="stats", bufs=1) as spool,
            tc.tile_pool(name="inp", bufs=4) as ipool,
        ):
            # consts via SWDGE so the SP HWDGE ring starts with bulk data
            w_sb = cpool.tile([128, 128], w_dt)
            nc.gpsimd.dma_start(out=w_sb[:], in_=w_const[:, :])
            id_sb = cpool.tile([128, 128], F32)
            nc.gpsimd.dma_start(out=id_sb[:], in_=id_const[:, :])
            d2 = spool.tile([128, TILES], F32)

            # [2048, 2048] -> super-tile view: s-th load is rows
            # [256s, 256s+256) laid out as [128 partitions, 2, 2048].
            emb_sv = emb[:, :].rearrange("(s a p) d -> s p a d", s=SUPER, a=2, p=128)

            def body(_iv=None):
                with tc.tile_pool(name="pdiff", bufs=2, space="PSUM") as pdiff:
                    for s_ in range(SUPER):
                        x = ipool.tile([128, 2 * D], in_dt)
                        x3 = x[:, :].rearrange("p (a d) -> p a d", a=2)
                        # alternate loads between the two independent DMA
                        # descriptor paths (HWDGE ring / SWDGE ring) so the
                        # per-transfer fixed costs overlap
                        if DMA_MODE == "hwdge":
                            eng = nc.sync
                        elif DMA_MODE == "swdge":
                            eng = nc.gpsimd
                        else:
                            eng = nc.sync if s_ % 2 == 0 else nc.gpsimd
                        eng.dma_start(out=x3, in_=emb_sv[s_])
                        if bf16_mm:
                            # truncated-bf16 view: upper 2 bytes of each f32
                            xmm = x[:, :].bitcast(mybir.dt.bfloat16).rearrange(
                                "p (n two) -> p n two", two=2
                            )[:, :, 1]
                        else:
                            xmm = x[:, :]
                        if stage == "dma":
                            # consume a sliver so the load isn't dead
                            nc.vector.tensor_copy(
                                d2[0:2, s_ : s_ + 1], x[0:2, 0:1].bitcast(F32)
                            )
                            continue
                        for a in range(2):
                            t = 2 * s_ + a
                            diff = pdiff.tile([128, D], F32)
                            nmm = 2 if stage == "mm2" else 4
                            for n in range(nmm):
                                rhs = xmm[:, a * D + 512 * n : a * D + 512 * (n + 1)]
                                nc.tensor.matmul(
                                    diff[:, 512 * n : 512 * (n + 1)],
                                    w_sb[:],
                                    rhs,
                                    start=True,
                                    stop=True,
                                )
                            if stage in ("mm", "mm2"):
                                nc.vector.tensor_copy(
                                    d2[0:2, t : t + 1], diff[0:2, 0:1]
                                )
                                continue
                            # d2[:, t] = sum_d diff^2 ; diff squared in place.
                            nc.scalar.activation(
                                diff[:], diff[:], AF.Square, accum_out=d2[:, t : t + 1]
                            )
                _tail()

            def _tail():
                with (
                    tc.tile_pool(name="ptail", bufs=1, space="PSUM") as ptail,
                    tc.tile_pool(name="tail", bufs=1) as tpool,
                ):
                    d2t = ptail.tile([TILES, 128], F32)
                    nc.tensor.transpose(d2t[:], d2[:], id_sb[:])
                    dist = tpool.tile([TILES, 128], F32)
                    nc.scalar.activation(dist[:], d2t[:], AF.Sqrt)
                    if NEWTON:
                        # dist <- 0.5 * (dist + d2 / dist)
                        rcp = tpool.tile([TILES, 128], F32)
                        nc.vector.reciprocal(rcp[:], dist[:])
                        quo = tpool.tile([TILES, 128], F32)
                        nc.vector.tensor_tensor(quo[:], d2t[:], rcp[:], op=ALU.mult)
                        tot = tpool.tile([TILES, 128], F32)
                        nc.vector.tensor_tensor(tot[:], dist[:], quo[:], op=ALU.add)
                        dist2 = tpool.tile([TILES, 128], F32)
                        nc.vector.tensor_scalar_mul(dist2[:], tot[:], 0.5)
                        dst = dist2
                    else:
                        dst = dist

                    losses = tpool.tile([TILES, 2], F32)
                    for j in range(2):
                        pos = dst[:, 64 * j : 64 * j + S]
                        neg = dst[:, 64 * j + S : 64 * j + 2 * S]
                        an = tpool.tile([TILES, 1], F32, tag=f"an{j}")
                        nc.vector.tensor_reduce(
                            an[:], neg, axis=mybir.AxisListType.X, op=ALU.min
                        )
                        anm = tpool.tile([TILES, 1], F32, tag=f"anm{j}")
                        nc.vector.tensor_scalar(
                            anm[:], an[:], float(margin), None, op0=ALU.subtract
                        )
                        hinge = tpool.tile([TILES, S], F32, tag=f"hinge{j}")
                        nc.vector.tensor_scalar(
                            hinge[:], pos, anm[:], 0.0, op0=ALU.subtract, op1=ALU.max
                        )
                        sq = tpool.tile([TILES, S], F32, tag=f"sq{j}")
                        nc.vector.tensor_tensor(sq[:], hinge[:], hinge[:], op=ALU.mult)
                        nc.vector.tensor_reduce(
                            losses[:, j : j + 1],
                            sq[:],
                            axis=mybir.AxisListType.X,
                            op=ALU.add,
                        )
                    nc.sync.dma_start(out=out[:, :], in_=losses[:])

            if loop_n is None:
                body()
            else:
                with tc.For_i(0, loop_n, 1):
                    body()

    nc.compile()
    return nc


def kernel(embeddings, target=None, margin=0.3, n_classes=256, n_samples=32, **_):
    global LAST_RESULTS
    emb = np.ascontiguousarray(np.asarray(embeddings, dtype=np.float32))
    assert emb.shape == (16384, 2048), emb.shape
    assert int(n_classes) == 256 and int(n_samples) == 32

    key = (float(margin), USE_FP32R, NEWTON, MM_DTYPE)
    nc = _CACHE.get(key)
    if nc is None:
        nc = _CACHE[key] = _build(float(margin))

    shards = emb.reshape(N_CORES, ROWS_PER_CORE, D)
    in_maps = [{"emb": shards[c]} for c in range(N_CORES)]
    res = run_bass_kernel_spmd(
        nc, in_maps, core_ids=list(range(N_CORES)), trace=TRACE
    )
    LAST_RESULTS = res
    per_class = np.concatenate(
        [r["losses"].reshape(-1) for r in res.results]
    )  # class order: core-major, then (tile, j) -> 2t + j: natural order
    return np.float32(per_class.mean())

